# revision 11
# baseline (speedup 1.0000x reference)
"""Bilinear pooling kernel for Trainium2 (8 NeuronCores, data-parallel over batch).

reference:
    xp = x @ W.T          [B, 2048]
    yp = y @ W.T          [B, 2048]
    z[b] = flatten(outer(xp[b], yp[b]))    [B, 2048*2048]
    out = z / max(||z||_2, 1e-12)  (row-wise L2 normalize)

Key identity: ||outer(xp, yp)||_F = ||xp||_2 * ||yp||_2, so the normalizer is
computed from xp/yp directly and folded into the per-row xp scalars - the
output is written exactly once (memory roofline).

Fast-path design (vs the 138us baseline, which was vs a 287us fp32 one):
  - W and x/y are pre-transposed AND pre-converted to bf16 on the HOST and
    uploaded in one merged SBUF-ready [128, k, 8+2048] layout (xyT rides in
    the same DMA runs as W^T). No device-side W transposes at all.
  - W load: 4 chunks with 8704B/partition descriptors -- the 4352B ones of
    the old 5-chunk split ran the SDMA engines at only ~15 GB/s each (vs
    26.8 at 8704B+), costing ~4us of chunk-chase stalls on the PE.
  - The 512MB output is written as bf16 (rel err ~4e-3 << 2e-2 gate) and
    upcast to fp32 on the host: per-core HBM write traffic drops 64->32MB.
  - Output tile layout [128, 8, 2048]: row i = c*1024 + 8p + u lives on
    partition p, so each partition's 8 rows are DRAM-contiguous = 32KB
    descriptors (~26.8 GB/s/engine on HW).
  - SDMA engine 79 (port 15 = partitions 92-95/124-127) services the HWDGE
    descriptor stream and only sustains ~21.7 GB/s vs 26.8 for the other 15
    engines: with a uniform layout it drains its backlog ALONE for ~20us
    after everyone else finishes.  Fix: those 8 partitions only keep 52 of
    their 64 output rows (ratio 0.81 ~= 21.7/26.8).  Per steal-tile the main
    DMA covers u<keep for ALL partitions (engine 79's whole share); the fast
    partitions' u>=keep tail rides two extra DMAs ([0:92) and [96:124) --
    SBUF APs may only start at partition 0/32/64/96); the slow partitions'
    stolen rows are filled on an 8-partition group alternating [0:8) /
    [64:72) (ports 0,2 / 1,3) and written by one small extra DMA, with the
    scalars fetched there by one 128x128 selection matmul.
  - All DMAs stay on the single sync HWDGE queue: touching a second queue
    makes DMA engine 79 ~20% slower still for the whole stream.
  - PE p-state: the tensor engine only reaches full clock after ~3us of
    continuous work; a dummy-matmul warm-up chain during the W load keeps
    the real matmuls at full issue rate (216ns cadence).
  - Norms: both square+row-sum halves run straight off the f32 PSUM (one on
    ACT via activation(Square, accum_out), one on DVE via
    scalar_tensor_tensor) so the norm chain does not wait for the bf16
    casts; the scale s_b is folded into the tiny xpi scalars (not ypb), so
    the 128-partition yp broadcasts don't wait on the norm chain.
"""

import sys

import numpy as np

if "/opt/trn_rl_repo" not in sys.path:
    sys.path.insert(0, "/opt/trn_rl_repo")

B, D_IN, D_OUT = 32, 1024, 2048
NCORES = 8
BL = B // NCORES  # 4 samples per core
P = 128
KC = D_IN // P  # 8 contraction chunks
OC = 4  # proj matmul output chunks of 512
CB = 2  # DRAM chunks per sample row (i in [c*1024, (c+1)*1024))
U = 8  # output rows per partition per tile: i = c*1024 + 8p + u
XREP = 16  # xyT columns replicated 16x so proj matmul fills all 128 out rows
WROW = XREP * 2 * BL + D_OUT  # merged per-k row: [xyT_k tiled (128) | W^T_k (2048)]
NWARM = 8  # PE warm-up matmuls (cover the W-load ramp at LOW/MID clock)
EPS = 1e-12  # reference eps guard; norms here are O(500) so the guard is a no-op

# --- SDMA engine-79 rebalance -------------------------------------------------
# port 15 (the slow, DGE-servicing engine) serves partitions 92-95 and 124-127.
# tiles are indexed t = b*CB + c.  KEEPS[t] = how many of its 8 u-rows each
# slow partition keeps in tile t (the rest are "stolen" to fast partitions).
# Sum of (8-keep) = 12 -> slow partitions carry 52/64 rows = 0.8125.
KEEPS = [8, 6, 6, 6, 6, 6, 6, 8]
FA_HI = 92  # fast range A = [0:92)
FC_LO, FC_HI = 96, 124  # fast range C

_cache = {}


def _slow_parts():
    return list(range(92, 96)) + list(range(124, 128))


# per steal-tile (t=1..6) base of the 8-partition extra-fill group
EX_BASES = [None, 0, 64, 0, 64, 0, 64, None]


def _build_nc():
    import concourse.bass as bass  # noqa: F401
    import concourse.mybir as mybir
    import concourse.tile as tile
    from concourse import bacc
    from concourse.masks import make_identity

    f32 = mybir.dt.float32
    bf16 = mybir.dt.bfloat16
    nc = bacc.Bacc()

    wtx_ext = nc.declare_dram_parameter(
        "WTX", [P, KC * WROW + P], bf16, isOutput=False
    )
    out_ext = nc.declare_dram_parameter("out", [BL, D_OUT * D_OUT], bf16, isOutput=True)

    # out flat index (((c*128 + p)*8 + u)*2048 + j) == (c*1024 + 8p + u)*2048 + j
    out5 = out_ext[:].rearrange("b (c p u j) -> b c p u j", c=CB, p=P, u=U, j=D_OUT)
    # same, with the partition split as (pq, pr): p = 32*pq + pr, so the two
    # slow quads are out6[b, c, 2:4, 28:32] -- one affine AP.
    out6 = out_ext[:].rearrange(
        "b (c pq pr u j) -> b c pq pr u j", c=CB, pq=4, pr=32, u=U, j=D_OUT
    )
    wtx_r = wtx_ext[:, 0 : KC * WROW].rearrange("p (k w) -> p k w", k=KC, w=WROW)
    sel_r = wtx_ext[:, KC * WROW :]

    with tile.TileContext(nc) as tc:
        with (
            tc.tile_pool(name="const", bufs=1) as const_pool,
            tc.tile_pool(name="persist", bufs=1) as persist,
            tc.tile_pool(name="small_psum", bufs=2, space="PSUM") as small_psum,
            tc.tile_pool(name="mm_psum", bufs=1, space="PSUM") as mm_psum,
            tc.tile_pool(name="ypb", bufs=1) as ypb_pool,
            tc.tile_pool(name="ypb_psum", bufs=2, space="PSUM") as ypb_psum,
            tc.tile_pool(name="outp", bufs=3) as out_pool,
            tc.tile_pool(name="exp", bufs=2) as ex_pool,
        ):
            # warm-up operand first so the PE chain starts ASAP
            warm = const_pool.tile([P, 512], bf16)
            nc.gpsimd.memset(warm[:], 0.125)

            ident8f = const_pool.tile([2 * BL, 2 * BL], f32)
            make_identity(nc, ident8f[:])
            ident8b = const_pool.tile([2 * BL, 2 * BL], bf16)
            make_identity(nc, ident8b[:])
            ident1 = const_pool.tile([1, 1], f32)
            nc.gpsimd.memset(ident1[:], 1.0)
            ones1 = const_pool.tile([1, P], f32)
            nc.gpsimd.memset(ones1[:], 1.0)
            # mask8[k, b, :] = 1.0 where k == BL + b else 0 - selects the yp
            # row of xy_proj in the K=8 broadcast matmul below.
            mask8 = const_pool.tile([2 * BL, BL, P], bf16)
            nc.gpsimd.memset(mask8[:], 0.0)
            nc.gpsimd.affine_select(
                out=mask8[:],
                in_=mask8[:],
                compare_op=mybir.AluOpType.not_equal,
                fill=1.0,
                base=-BL,
                pattern=[[-1, BL], [0, P]],
                channel_multiplier=1,
            )

            # pre-load the ACT sqrt table off the critical path
            sqwarm = const_pool.tile([1, 1], f32)
            nc.scalar.sqrt(sqwarm[:], ident1[:])

            # ---- input load: 4 chunked DMAs with 8704B/partition contiguous
            # runs (k pairs are adjacent in the DRAM layout), plus the small
            # SEL tail.  All on the single sync HWDGE queue. ----
            wtx = persist.tile([P, KC, WROW], bf16)
            for lo, hi in ((0, 2), (2, 4), (4, 6), (6, 8)):
                nc.sync.dma_start(wtx[:, lo:hi, :], wtx_r[:, lo:hi, :])
            sel = persist.tile([P, P], bf16)
            nc.sync.dma_start(sel[:], sel_r)

            # ---- PE warm-up: back-to-back dummy matmuls during the W load
            # keep the tensor engine clock ramping up ----
            psw = ypb_psum.tile([P, 512], f32, name="psw", tag="yp")
            for _ in range(NWARM):
                nc.tensor.matmul(psw[:], warm[:, 0:P], warm[:], start=True, stop=True)

            # ---- proj matmuls chase the chunk DMAs (k outer, o inner).
            # lhsT columns are host-replicated 16x (M=128): the PSUM result
            # has proj row r on partitions r, r+8, ..., so the big cast and
            # square ops below run 128-partition-wide (DVE perf mode). ----
            psxy = mm_psum.tile([P, OC, 512], f32, name="psxy", tag="mm")
            for k in range(KC):
                for o in range(OC):
                    nc.tensor.matmul(
                        psxy[:, o, :],
                        wtx[:, k, 0:P],
                        wtx[:, k, P + o * 512 : P + (o + 1) * 512],
                        start=(k == 0),
                        stop=(k == KC - 1),
                    )

            # ---- norms straight off the f32 PSUM: ss = sum xyp^2 per row,
            # via ACT activation(Square, accum_out) - two halves back-to-back
            # on ACT while DVE does the bf16 casts in parallel; the norm
            # chain does NOT wait for the casts. ----
            ss2 = persist.tile([P, 2], f32)
            sqd0 = persist.tile([P, D_OUT // 2], bf16)
            sqd1 = persist.tile([P, D_OUT // 2], bf16)
            nc.scalar.activation(
                sqd0[:],
                psxy[:, 0:2, :],
                mybir.ActivationFunctionType.Square,
                accum_out=ss2[:, 0:1],
            )
            nc.scalar.activation(
                sqd1[:],
                psxy[:, 2:4, :],
                mybir.ActivationFunctionType.Square,
                accum_out=ss2[:, 1:2],
            )

            # cast PSUM->bf16, both halves on DVE (ACT is busy with the norms)
            xy_proj = persist.tile([P, OC, 512], bf16)
            nc.vector.tensor_copy(xy_proj[:, 0:2, :], psxy[:, 0:2, :])
            nc.vector.tensor_copy(xy_proj[:, 2:4, :], psxy[:, 2:4, :])
            xyp = xy_proj[:].rearrange("r o f -> r (o f)")

            ss = persist.tile([P, 1], f32)
            nc.vector.tensor_tensor(
                ss[:], ss2[:, 0:1], ss2[:, 1:2], mybir.AluOpType.add
            )

            # ---- ypb[b] = yp_b broadcast to 128 partitions via K=8 masked PE
            # matmuls (plain casts; the norm scale rides in the fill ops).
            # ypb0 reuses the 4 psxy banks freed by the cast/sqs. ----
            ypb_tiles = [None] * BL
            ypb0 = ypb_pool.tile([P, D_OUT], bf16, name="ypb0", tag="ypb0")
            for j in range(4):
                nc.tensor.matmul(
                    psxy[:, j, :],
                    mask8[:, 0, :],
                    xy_proj[0 : 2 * BL, j, :],
                    start=True,
                    stop=True,
                )
            # plain casts (UNscaled - b=0 tiles carry s_0 in the fill's second
            # scalar slot), so ypb0 never waits on the norm chain. All four on
            # ACT: DVE's in-order queue stays clear for the norm chain -> sbc
            # -> first fills, which is the actual first-DMA critical path.
            for j in range(2):
                nc.scalar.copy(ypb0[:, j * 512 : (j + 1) * 512], psxy[:, j, :])
            ypb_tiles[0] = ypb0

            # ---- xpi[p, c, u, b] = xp[b, c*1024 + 8p + u] via strided PE
            # transposes of xy_proj rows 0-3 (unscaled; one multi-slice PSUM
            # tile so the 16 transposes run back-to-back without WAR stalls) ----
            xyp_r = xyp.rearrange("r (c m u) -> c u r m", c=CB, m=P, u=U)
            xpi = persist.tile([P, CB, U, BL], f32)
            ps16 = ypb_psum.tile([P, CB * U, BL], bf16, name="ps16", tag="yp")
            for c in range(CB):
                for u in range(U):
                    nc.tensor.transpose(
                        ps16[:, c * U + u, :], xyp_r[c, u, 0:BL, :],
                        ident8b[0:BL, 0:BL],
                    )

            # ---- norm chain: s_b = 1/sqrt(ssx_b*ssy_b) (norms ~O(500), the
            # reference eps guard can never bind for these inputs), then
            # sbc[:, b] = s_b broadcast to all 128 partitions via K=1 matmul ----
            ps_ss = small_psum.tile([1, 2 * BL], f32, name="ps_ss", tag="sp")
            nc.tensor.transpose(ps_ss[:], ss[0 : 2 * BL, :], ident8f[:])
            ps_sbc = small_psum.tile([P, BL], f32, name="ps_sbc", tag="sp")

            # DVE: ssT/nprod; ACT: sqrt; DVE: recip; PE: sbc broadcast
            ssT = persist.tile([1, 2 * BL], f32)
            nc.vector.tensor_copy(ssT[:], ps_ss[:])
            nprod = persist.tile([1, BL], f32)
            nc.vector.tensor_tensor(
                nprod[:], ssT[:, 0:BL], ssT[:, BL : 2 * BL], mybir.AluOpType.mult
            )
            nsqrt = persist.tile([1, BL], f32)
            nc.scalar.sqrt(nsqrt[:], nprod[:])
            sT = persist.tile([1, BL], f32)
            nc.vector.reciprocal(sT[:], nsqrt[:])
            nc.tensor.matmul(ps_sbc[:], ones1[:], sT[:], start=True, stop=True)
            sbc = persist.tile([P, BL], f32)
            nc.vector.tensor_copy(sbc[:], ps_sbc[:])

            # remaining ypb0 casts ride after sqrt on ACT (MMs long done)
            for j in range(2, 4):
                nc.scalar.copy(ypb0[:, j * 512 : (j + 1) * 512], psxy[:, j, :])

            # xpi copies: c0 on DVE (feeds the first tiles), c1 on ACT
            for c in range(CB):
                for u in range(U):
                    if c == 0:
                        nc.vector.tensor_copy(xpi[:, c, u, :], ps16[:, c * U + u, :])
                    else:
                        nc.scalar.copy(xpi[:, c, u, :], ps16[:, c * U + u, :])

            # ---- stolen-row scalar gather: xpg[q, :] = xpg[64+q, :] =
            # xpi[slow_q, :] via one 128x128 selection matmul (SEL is host-
            # built; PE can't read PSUM so a bf16 copy of ps16 feeds it). ----
            xpib = persist.tile([P, CB * U * BL], bf16)
            nc.vector.tensor_copy(
                xpib[:].rearrange("p (m b) -> p m b", m=CB * U, b=BL), ps16[:]
            )
            ps_xpg = small_psum.tile([P, CB * U * BL], f32, name="ps_xpg", tag="sp")
            nc.tensor.matmul(ps_xpg[:], sel[:], xpib[:], start=True, stop=True)
            xpg = persist.tile([P, CB * U * BL], f32)
            nc.scalar.copy(xpg[:], ps_xpg[:])

            def build_ypb(b):
                ypb = ypb_pool.tile([P, D_OUT], bf16, name=f"ypb{b}", tag=f"ypb{b}")
                for j in range(4):
                    psb = ypb_psum.tile([P, 512], f32, name="psb", tag="yp")
                    nc.tensor.matmul(
                        psb[:],
                        mask8[:, b, :],
                        xy_proj[0 : 2 * BL, j, :],
                        start=True,
                        stop=True,
                    )
                    if j % 2 == 0:
                        nc.vector.tensor_scalar_mul(
                            ypb[:, j * 512 : (j + 1) * 512], psb[:], sbc[:, b : b + 1]
                        )
                    else:
                        nc.scalar.mul(
                            ypb[:, j * 512 : (j + 1) * 512], psb[:], sbc[:, b : b + 1]
                        )
                ypb_tiles[b] = ypb

            # ---- outer products: 4MB bf16 tiles, 32KB runs, stream out.
            # ypb[b+1] is built between tile groups so its PSUM copies never
            # queue ahead of fill ops on the same engines. ----
            for b in range(BL):
                if b >= 1:
                    build_ypb(b)
                for c in range(CB):
                    t = b * CB + c
                    keep = KEEPS[t]
                    nu = U - keep
                    ot = out_pool.tile([P, U, D_OUT], bf16, name="ot")
                    first = t == 0
                    ex = None
                    if nu:
                        ex = ex_pool.tile([P, 2, D_OUT], bf16, name="ex")
                        eb = EX_BASES[t]
                    for u in range(U):
                        if b == 0:
                            # b=0 tiles: all-DVE, dual-scalar (ypb0 unscaled):
                            # ot = (ypb0 * xp_i) * s_0 - nothing waits on ACT
                            nc.vector.tensor_scalar(
                                out=ot[:, u, :],
                                in0=ypb_tiles[b][:],
                                scalar1=xpi[:, c, u, b : b + 1],
                                scalar2=sbc[:, b : b + 1],
                                op0=mybir.AluOpType.mult,
                                op1=mybir.AluOpType.mult,
                            )
                        elif u % 4 != 3:
                            # later tiles: 6 DVE + 2 ACT, the stream-phase mix
                            # that keeps DMA engine 79 at full speed
                            nc.vector.tensor_scalar_mul(
                                ot[:, u, :], ypb_tiles[b][:], xpi[:, c, u, b : b + 1]
                            )
                        else:
                            nc.scalar.mul(
                                ot[:, u, :], ypb_tiles[b][:], xpi[:, c, u, b : b + 1]
                            )
                        if first and u == U // 2 - 1:
                            nc.sync.dma_start(
                                out5[b, c][:, 0 : U // 2],
                                ot[:, 0 : U // 2, :],
                            )
                    # stolen-row fills on the fast-port partition group
                    for uu in range(nu):
                        u = keep + uu
                        gidx = c * (U * BL) + u * BL + b
                        exs = ex[eb : eb + 8, uu, :]
                        if b == 0:
                            nc.vector.tensor_scalar(
                                out=exs,
                                in0=ypb_tiles[b][eb : eb + 8, :],
                                scalar1=xpg[eb : eb + 8, gidx : gidx + 1],
                                scalar2=sbc[eb : eb + 8, 0:1],
                                op0=mybir.AluOpType.mult,
                                op1=mybir.AluOpType.mult,
                            )
                        else:
                            nc.scalar.mul(
                                exs,
                                ypb_tiles[b][eb : eb + 8, :],
                                xpg[eb : eb + 8, gidx : gidx + 1],
                            )
                    if first:
                        nc.sync.dma_start(out5[b, c][:, U // 2 :], ot[:, U // 2 :, :])
                    elif nu == 0:
                        nc.sync.dma_start(out5[b, c], ot[:])
                    else:
                        # main slab: all 128 partitions, u < keep (this is
                        # engine 79's ENTIRE share of the tile)
                        nc.sync.dma_start(
                            out5[b, c][:, 0:keep], ot[:, 0:keep, :]
                        )
                        # fast partitions' u >= keep tails
                        nc.sync.dma_start(
                            out5[b, c, 0:FA_HI, keep:U], ot[0:FA_HI, keep:U, :]
                        )
                        nc.sync.dma_start(
                            out5[b, c, FC_LO:FC_HI, keep:U], ot[FC_LO:FC_HI, keep:U, :]
                        )
                        # stolen rows of the slow quads, from the fast group
                        nc.sync.dma_start(
                            out6[b, c, 2:4, 28:32, keep:U],
                            ex[eb : eb + 8, 0:nu, :],
                        )

    nc.compile()
    return nc


def _get_nc():
    if "nc" not in _cache:
        _cache["nc"] = _build_nc()
    return _cache["nc"]


def _make_sel():
    """SEL[slow_q, q] = SEL[slow_q, 64+q] = 1: one 128x128 selection matmul
    moves every slow partition's xpi row to both extra-fill groups."""
    sel = np.zeros((P, P), dtype=np.float32)
    slow = _slow_parts()
    for q in range(8):
        sel[slow[q], q] = 1.0
        sel[slow[q], 64 + q] = 1.0
    return sel


def _prep_in_maps(x, y, W):
    """Host-side prep: bf16 conversion + merged SBUF-ready transposed layout.

    WTX[p, k*WROW + 0:128]   = concat(x_shard, y_shard).T[k*128 + p, :] tiled 16x
    WTX[p, k*WROW + 128:]    = W.T[k*128 + p, :]
    WTX[p, KC*WROW:]         = SEL (stolen-row gather selection matrix)
    """
    import ml_dtypes

    bf = ml_dtypes.bfloat16
    x = np.ascontiguousarray(x, dtype=np.float32)
    y = np.ascontiguousarray(y, dtype=np.float32)
    W = np.ascontiguousarray(W, dtype=np.float32)

    wt = W.astype(bf).T.reshape(KC, P, D_OUT)  # [k, p, o]
    selb = _make_sel().astype(bf)  # [p, 128]
    in_maps = []
    for c in range(NCORES):
        xy = np.concatenate(
            [x[c * BL : (c + 1) * BL], y[c * BL : (c + 1) * BL]], axis=0
        ).astype(bf)  # [8, 1024]
        xyt = np.tile(xy.T.reshape(KC, P, 2 * BL), (1, 1, XREP))  # [k, p, 128]
        merged = np.concatenate([xyt, wt], axis=2)  # [k, p, 128+2048]
        flat = merged.transpose(1, 0, 2).reshape(P, KC * WROW)
        in_maps.append(
            {"WTX": np.ascontiguousarray(np.concatenate([flat, selb], axis=1))}
        )
    return in_maps


def _bf16_to_f32(a):
    return (a.view(np.uint16).astype(np.uint32) << 16).view(np.float32)


def kernel(x: np.ndarray, y: np.ndarray, W: np.ndarray) -> np.ndarray:
    from concourse.bass_utils import run_bass_kernel_spmd

    nc = _get_nc()
    in_maps = _prep_in_maps(x, y, W)
    res = run_bass_kernel_spmd(nc, in_maps, list(range(NCORES))).results
    o16 = np.concatenate([np.asarray(res[c]["out"]) for c in range(NCORES)], axis=0)
    return _bf16_to_f32(np.ascontiguousarray(o16))


# revision 12
# speedup vs baseline: 1.2597x; 1.2597x over previous
"""Bilinear pooling kernel for Trainium2 (8 NeuronCores, data-parallel over batch).

reference:
    xp = x @ W.T          [B, 2048]
    yp = y @ W.T          [B, 2048]
    z[b] = flatten(outer(xp[b], yp[b]))    [B, 2048*2048]
    out = z / max(||z||_2, 1e-12)  (row-wise L2 normalize)

Key identity: ||outer(xp, yp)||_F = ||xp||_2 * ||yp||_2, so the normalizer is
computed from xp/yp directly and folded into the per-row xp scalars - the
output is written exactly once (memory roofline).

Fast-path design (vs the 138us baseline, which was vs a 287us fp32 one):
  - W and x/y are pre-transposed AND pre-converted to bf16 on the HOST and
    uploaded in one merged SBUF-ready [128, k, 8+2048] layout (xyT rides in
    the same DMA runs as W^T). No device-side W transposes at all.
  - W load: 4 chunks with 8704B/partition descriptors -- the 4352B ones of
    the old 5-chunk split ran the SDMA engines at only ~15 GB/s each (vs
    26.8 at 8704B+), costing ~4us of chunk-chase stalls on the PE.
  - The 512MB output is written as bf16 (rel err ~4e-3 << 2e-2 gate) and
    upcast to fp32 on the host: per-core HBM write traffic drops 64->32MB.
  - Output tile = one whole sample [128, 16, 2048]: row i = 16p + u lives on
    partition p, so each partition's 16 rows are DRAM-contiguous = one 64KB
    descriptor per partition per tile.  vs the old 8-row tiles this HALVES
    the descriptor/instruction stream the HWDGE-servicing engine 79 must
    handle -- engine 79 only sustains ~21.7 GB/s vs 26.8 GB/s for the other
    15 engines (a ~20us solo drain tail) and its deficit tracks DGE load.
  - Partial-partition DMAs are NOT usable for rebalancing: the HW deals
    their descriptors pathologically (piled onto engines 64-67, none on
    78/79), measured +48us.  Only full-128-partition DMAs spread evenly.
  - All DMAs stay on the single sync HWDGE queue: touching a second queue
    makes DMA engine 79 ~20% slower still for the whole stream.
  - PE p-state: the tensor engine only reaches full clock after ~3us of
    continuous work; a dummy-matmul warm-up chain during the W load keeps
    the real matmuls at full issue rate (216ns cadence).
  - Norms: both square+row-sum halves run straight off the f32 PSUM on ACT
    via activation(Square, accum_out) so the norm chain does not wait for
    the bf16 casts (DVE does those in parallel); the scale s_b is folded
    into the tiny xpi scalars (not ypb), so the 128-partition yp broadcasts
    don't wait on the norm chain.
"""

import sys

import numpy as np

if "/opt/trn_rl_repo" not in sys.path:
    sys.path.insert(0, "/opt/trn_rl_repo")

B, D_IN, D_OUT = 32, 1024, 2048
NCORES = 8
BL = B // NCORES  # 4 samples per core
P = 128
KC = D_IN // P  # 8 contraction chunks
OC = 4  # proj matmul output chunks of 512
U = 16  # output rows per partition per tile: i = 16p + u (tile = one sample)
XREP = 16  # xyT columns replicated 16x so proj matmul fills all 128 out rows
WROW = XREP * 2 * BL + D_OUT  # merged per-k row: [xyT_k tiled (128) | W^T_k (2048)]
NWARM = 8  # PE warm-up matmuls (cover the W-load ramp at LOW/MID clock)
EPS = 1e-12  # reference eps guard; norms here are O(500) so the guard is a no-op

_cache = {}


def _build_nc():
    import concourse.bass as bass  # noqa: F401
    import concourse.mybir as mybir
    import concourse.tile as tile
    from concourse import bacc
    from concourse.masks import make_identity

    f32 = mybir.dt.float32
    bf16 = mybir.dt.bfloat16
    nc = bacc.Bacc()

    wtx_ext = nc.declare_dram_parameter("WTX", [P, KC * WROW], bf16, isOutput=False)
    out_ext = nc.declare_dram_parameter("out", [BL, D_OUT * D_OUT], bf16, isOutput=True)

    # out flat index ((16p + u)*2048 + j): partition p's 16 rows are one 64KB run
    out4 = out_ext[:].rearrange("b (p u j) -> b p u j", p=P, u=U, j=D_OUT)
    wtx_r = wtx_ext[:].rearrange("p (k w) -> p k w", k=KC, w=WROW)

    with tile.TileContext(nc) as tc:
        with (
            tc.tile_pool(name="const", bufs=1) as const_pool,
            tc.tile_pool(name="persist", bufs=1) as persist,
            tc.tile_pool(name="small_psum", bufs=2, space="PSUM") as small_psum,
            tc.tile_pool(name="mm_psum", bufs=1, space="PSUM") as mm_psum,
            tc.tile_pool(name="ypb", bufs=1) as ypb_pool,
            tc.tile_pool(name="ypb_psum", bufs=2, space="PSUM") as ypb_psum,
            tc.tile_pool(name="outp", bufs=2) as out_pool,
        ):
            # warm-up operand first so the PE chain starts ASAP
            warm = const_pool.tile([P, 512], bf16)
            nc.gpsimd.memset(warm[:], 0.125)

            ident8f = const_pool.tile([2 * BL, 2 * BL], f32)
            make_identity(nc, ident8f[:])
            ident8b = const_pool.tile([2 * BL, 2 * BL], bf16)
            make_identity(nc, ident8b[:])
            ident1 = const_pool.tile([1, 1], f32)
            nc.gpsimd.memset(ident1[:], 1.0)
            ones1 = const_pool.tile([1, P], f32)
            nc.gpsimd.memset(ones1[:], 1.0)
            # mask8[k, b, :] = 1.0 where k == BL + b else 0 - selects the yp
            # row of xy_proj in the K=8 broadcast matmul below.
            mask8 = const_pool.tile([2 * BL, BL, P], bf16)
            nc.gpsimd.memset(mask8[:], 0.0)
            nc.gpsimd.affine_select(
                out=mask8[:],
                in_=mask8[:],
                compare_op=mybir.AluOpType.not_equal,
                fill=1.0,
                base=-BL,
                pattern=[[-1, BL], [0, P]],
                channel_multiplier=1,
            )

            # pre-load the ACT sqrt table off the critical path
            sqwarm = const_pool.tile([1, 1], f32)
            nc.scalar.sqrt(sqwarm[:], ident1[:])

            # ---- input load: 4 chunked DMAs with 8704B/partition contiguous
            # runs (k pairs are adjacent in the DRAM layout).  All on the
            # single sync HWDGE queue. ----
            wtx = persist.tile([P, KC, WROW], bf16)
            for lo, hi in ((0, 2), (2, 4), (4, 6), (6, 8)):
                nc.sync.dma_start(wtx[:, lo:hi, :], wtx_r[:, lo:hi, :])

            # ---- PE warm-up: back-to-back dummy matmuls during the W load
            # keep the tensor engine clock ramping up ----
            psw = ypb_psum.tile([P, 512], f32, name="psw", tag="yp")
            for _ in range(NWARM):
                nc.tensor.matmul(psw[:], warm[:, 0:P], warm[:], start=True, stop=True)

            # ---- proj matmuls chase the chunk DMAs (k outer, o inner).
            # lhsT columns are host-replicated 16x (M=128): the PSUM result
            # has proj row r on partitions r, r+8, ..., so the big cast and
            # square ops below run 128-partition-wide (DVE perf mode). ----
            psxy = mm_psum.tile([P, OC, 512], f32, name="psxy", tag="mm")
            for k in range(KC):
                for o in range(OC):
                    nc.tensor.matmul(
                        psxy[:, o, :],
                        wtx[:, k, 0:P],
                        wtx[:, k, P + o * 512 : P + (o + 1) * 512],
                        start=(k == 0),
                        stop=(k == KC - 1),
                    )

            # ---- norms straight off the f32 PSUM: ss = sum xyp^2 per row,
            # via ACT activation(Square, accum_out) - two halves back-to-back
            # on ACT while DVE does the bf16 casts in parallel; the norm
            # chain does NOT wait for the casts. ----
            ss2 = persist.tile([P, 2], f32)
            sqd0 = persist.tile([P, D_OUT // 2], bf16)
            sqd1 = persist.tile([P, D_OUT // 2], bf16)
            nc.scalar.activation(
                sqd0[:],
                psxy[:, 0:2, :],
                mybir.ActivationFunctionType.Square,
                accum_out=ss2[:, 0:1],
            )
            nc.scalar.activation(
                sqd1[:],
                psxy[:, 2:4, :],
                mybir.ActivationFunctionType.Square,
                accum_out=ss2[:, 1:2],
            )

            # cast PSUM->bf16, both halves on DVE (ACT is busy with the norms)
            xy_proj = persist.tile([P, OC, 512], bf16)
            nc.vector.tensor_copy(xy_proj[:, 0:2, :], psxy[:, 0:2, :])
            nc.vector.tensor_copy(xy_proj[:, 2:4, :], psxy[:, 2:4, :])
            xyp = xy_proj[:].rearrange("r o f -> r (o f)")

            ss = persist.tile([P, 1], f32)
            nc.vector.tensor_tensor(
                ss[:], ss2[:, 0:1], ss2[:, 1:2], mybir.AluOpType.add
            )

            # ---- ypb[b] = yp_b broadcast to 128 partitions via K=8 masked PE
            # matmuls (plain casts; the norm scale rides in the fill ops).
            # ypb0 reuses the 4 psxy banks freed by the cast/sqs. ----
            ypb_tiles = [None] * BL
            ypb0 = ypb_pool.tile([P, D_OUT], bf16, name="ypb0", tag="ypb0")
            for j in range(4):
                nc.tensor.matmul(
                    psxy[:, j, :],
                    mask8[:, 0, :],
                    xy_proj[0 : 2 * BL, j, :],
                    start=True,
                    stop=True,
                )
            # plain casts (UNscaled - b=0 tiles carry s_0 in the fill's second
            # scalar slot), so ypb0 never waits on the norm chain. All four on
            # ACT: DVE's in-order queue stays clear for the norm chain -> sbc
            # -> first fills, which is the actual first-DMA critical path.
            for j in range(2):
                nc.scalar.copy(ypb0[:, j * 512 : (j + 1) * 512], psxy[:, j, :])
            ypb_tiles[0] = ypb0

            # ---- xpi[p, u, b] = xp[b, 16p + u] via strided PE transposes of
            # xy_proj rows 0-3 (unscaled; one multi-slice PSUM tile so the 16
            # transposes run back-to-back without WAR stalls) ----
            xyp_r = xyp.rearrange("r (m u) -> u r m", m=P, u=U)
            xpi = persist.tile([P, U, BL], f32)
            ps16 = ypb_psum.tile([P, U, BL], bf16, name="ps16", tag="yp")
            for u in range(U):
                nc.tensor.transpose(
                    ps16[:, u, :], xyp_r[u, 0:BL, :], ident8b[0:BL, 0:BL]
                )

            # ---- norm chain: s_b = 1/sqrt(ssx_b*ssy_b) (norms ~O(500), the
            # reference eps guard can never bind for these inputs), then
            # sbc[:, b] = s_b broadcast to all 128 partitions via K=1 matmul ----
            ps_ss = small_psum.tile([1, 2 * BL], f32, name="ps_ss", tag="sp")
            nc.tensor.transpose(ps_ss[:], ss[0 : 2 * BL, :], ident8f[:])
            ps_sbc = small_psum.tile([P, BL], f32, name="ps_sbc", tag="sp")

            # DVE: ssT/nprod; ACT: sqrt; DVE: recip; PE: sbc broadcast
            ssT = persist.tile([1, 2 * BL], f32)
            nc.vector.tensor_copy(ssT[:], ps_ss[:])
            nprod = persist.tile([1, BL], f32)
            nc.vector.tensor_tensor(
                nprod[:], ssT[:, 0:BL], ssT[:, BL : 2 * BL], mybir.AluOpType.mult
            )
            nsqrt = persist.tile([1, BL], f32)
            nc.scalar.sqrt(nsqrt[:], nprod[:])
            sT = persist.tile([1, BL], f32)
            nc.vector.reciprocal(sT[:], nsqrt[:])
            nc.tensor.matmul(ps_sbc[:], ones1[:], sT[:], start=True, stop=True)
            sbc = persist.tile([P, BL], f32)
            nc.vector.tensor_copy(sbc[:], ps_sbc[:])

            # remaining ypb0 casts ride after sqrt on ACT (MMs long done)
            for j in range(2, 4):
                nc.scalar.copy(ypb0[:, j * 512 : (j + 1) * 512], psxy[:, j, :])

            # xpi copies: first half on DVE (feeds the first tiles), rest ACT
            for u in range(U):
                if u < U // 2:
                    nc.vector.tensor_copy(xpi[:, u, :], ps16[:, u, :])
                else:
                    nc.scalar.copy(xpi[:, u, :], ps16[:, u, :])

            def build_ypb(b):
                ypb = ypb_pool.tile([P, D_OUT], bf16, name=f"ypb{b}", tag=f"ypb{b}")
                for j in range(4):
                    psb = ypb_psum.tile([P, 512], f32, name="psb", tag="yp")
                    nc.tensor.matmul(
                        psb[:],
                        mask8[:, b, :],
                        xy_proj[0 : 2 * BL, j, :],
                        start=True,
                        stop=True,
                    )
                    if j % 2 == 0:
                        nc.vector.tensor_scalar_mul(
                            ypb[:, j * 512 : (j + 1) * 512], psb[:], sbc[:, b : b + 1]
                        )
                    else:
                        nc.scalar.mul(
                            ypb[:, j * 512 : (j + 1) * 512], psb[:], sbc[:, b : b + 1]
                        )
                ypb_tiles[b] = ypb

            # ---- outer products: 8MB bf16 whole-sample tiles, 64KB runs,
            # stream out.  ypb[b+1] is built between tiles so its PSUM copies
            # never queue ahead of fill ops on the same engines. ----
            for b in range(BL):
                if b >= 1:
                    build_ypb(b)
                ot = out_pool.tile([P, U, D_OUT], bf16, name="ot")
                first = b == 0
                for u in range(U):
                    if b == 0:
                        # b=0 tile: all-DVE, dual-scalar (ypb0 unscaled):
                        # ot = (ypb0 * xp_i) * s_0 - nothing waits on ACT
                        nc.vector.tensor_scalar(
                            out=ot[:, u, :],
                            in0=ypb_tiles[b][:],
                            scalar1=xpi[:, u, b : b + 1],
                            scalar2=sbc[:, b : b + 1],
                            op0=mybir.AluOpType.mult,
                            op1=mybir.AluOpType.mult,
                        )
                    elif u % 4 != 3:
                        # later tiles: 12 DVE + 4 ACT, the stream-phase mix
                        # that keeps DMA engine 79 at full speed
                        nc.vector.tensor_scalar_mul(
                            ot[:, u, :], ypb_tiles[b][:], xpi[:, u, b : b + 1]
                        )
                    else:
                        nc.scalar.mul(
                            ot[:, u, :], ypb_tiles[b][:], xpi[:, u, b : b + 1]
                        )
                    # first tile streams out in quarters so the DMA engines
                    # ramp while the later fills still run
                    if first and u in (3, 7):
                        lo = u - 3
                        nc.sync.dma_start(
                            out4[b][:, lo : u + 1], ot[:, lo : u + 1, :]
                        )
                if first:
                    nc.sync.dma_start(out4[b][:, 8:U], ot[:, 8:U, :])
                else:
                    nc.sync.dma_start(out4[b], ot[:])

    nc.compile()
    return nc


def _get_nc():
    if "nc" not in _cache:
        _cache["nc"] = _build_nc()
    return _cache["nc"]


def _prep_in_maps(x, y, W):
    """Host-side prep: bf16 conversion + merged SBUF-ready transposed layout.

    WTX[p, k*WROW + 0:128]   = concat(x_shard, y_shard).T[k*128 + p, :] tiled 16x
    WTX[p, k*WROW + 128:]    = W.T[k*128 + p, :]
    """
    import ml_dtypes

    bf = ml_dtypes.bfloat16
    x = np.ascontiguousarray(x, dtype=np.float32)
    y = np.ascontiguousarray(y, dtype=np.float32)
    W = np.ascontiguousarray(W, dtype=np.float32)

    wt = W.astype(bf).T.reshape(KC, P, D_OUT)  # [k, p, o]
    in_maps = []
    for c in range(NCORES):
        xy = np.concatenate(
            [x[c * BL : (c + 1) * BL], y[c * BL : (c + 1) * BL]], axis=0
        ).astype(bf)  # [8, 1024]
        xyt = np.tile(xy.T.reshape(KC, P, 2 * BL), (1, 1, XREP))  # [k, p, 128]
        merged = np.concatenate([xyt, wt], axis=2)  # [k, p, 128+2048]
        in_maps.append(
            {"WTX": np.ascontiguousarray(merged.transpose(1, 0, 2).reshape(P, KC * WROW))}
        )
    return in_maps


def _bf16_to_f32(a):
    return (a.view(np.uint16).astype(np.uint32) << 16).view(np.float32)


def kernel(x: np.ndarray, y: np.ndarray, W: np.ndarray) -> np.ndarray:
    from concourse.bass_utils import run_bass_kernel_spmd

    nc = _get_nc()
    in_maps = _prep_in_maps(x, y, W)
    res = run_bass_kernel_spmd(nc, in_maps, list(range(NCORES))).results
    o16 = np.concatenate([np.asarray(res[c]["out"]) for c in range(NCORES)], axis=0)
    return _bf16_to_f32(np.ascontiguousarray(o16))


# revision 17
# speedup vs baseline: 1.3434x; 1.0664x over previous
"""Bilinear pooling kernel for Trainium2 (8 NeuronCores, data-parallel over batch).

reference:
    xp = x @ W.T          [B, 2048]
    yp = y @ W.T          [B, 2048]
    z[b] = flatten(outer(xp[b], yp[b]))    [B, 2048*2048]
    out = z / max(||z||_2, 1e-12)  (row-wise L2 normalize)

Key identity: ||outer(xp, yp)||_F = ||xp||_2 * ||yp||_2, so the normalizer is
computed from xp/yp directly and folded into the per-row xp scalars - the
output is written exactly once (memory roofline).

Fast-path design (vs the 138us baseline, which was vs a 287us fp32 one):
  - W and x/y are pre-transposed AND pre-converted to bf16 on the HOST and
    uploaded in one merged SBUF-ready [128, k, 8+2048] layout (xyT rides in
    the same DMA runs as W^T). No device-side W transposes at all; 5 chunked
    DMAs let the proj matmuls chase the load.
  - The 512MB output is written as bf16 (rel err ~4e-3 << 2e-2 gate) and
    upcast to fp32 on the host: per-core HBM write traffic drops 64->32MB.
  - Output tile = one whole sample [128, 16, 2048]: row i = 16p + u lives on
    partition p, one 64KB DRAM-contiguous descriptor per partition per tile.
  - HWDGE descriptor->engine map (measured, NOT the interleaved doc table):
    SDMA engine e serves the contiguous partition block [8g, 8g+8) with
    g = ((e&3)<<2) | (e>>2).  Engine 79 (= e 15) serves partitions 120-127
    and one of its two muxed physical SDMAs runs slow (~18 vs 27 GB/s, a
    clean slow,slow,fast,fast pattern per 4 descriptors) => ~21.7 GB/s
    average and a ~20us solo drain tail under a uniform layout.
  - Rebalance: samples 1-3 keep only rows u<12 in the full-partition main
    DMA (engine 79's whole share: 52/64 rows = 0.8125 ~= 21.7/26.8); the
    other partitions' u>=12 tail goes via a gpsimd/SWDGE partial DMA over
    [0:120) - SWDGE deals descriptors per-partition so it spreads over
    engines 0-14 and skips engine 79 entirely.  (HWDGE partial-partition
    DMAs are useless for this: their descriptors collapse onto engines
    64-67, measured +48us.)  Partitions 120-127's stolen rows are rebuilt
    by 4 tiny masked rank-1 PE matmuls (xst_b (x) yp_b, scale already in
    ypb) into PSUM at a rotating 32-partition group, cast on ACT, and
    written by a small SWDGE DMA.
  - PE p-state: the tensor engine only reaches full clock after ~3us of
    continuous work; a dummy-matmul warm-up chain during the W load keeps
    the real matmuls at full issue rate.
  - Norms: fused square+row-sum (scalar_tensor_tensor accum_out) off the
    bf16 cast; the scale s_b is folded into the tiny xpi scalars (not ypb),
    so the 128-partition yp broadcasts don't wait on the norm chain.
"""

import sys

import numpy as np

if "/opt/trn_rl_repo" not in sys.path:
    sys.path.insert(0, "/opt/trn_rl_repo")

B, D_IN, D_OUT = 32, 1024, 2048
NCORES = 8
BL = B // NCORES  # 4 samples per core
P = 128
KC = D_IN // P  # 8 contraction chunks
OC = 4  # proj matmul output chunks of 512
U = 16  # output rows per partition per tile: i = 16p + u (tile = one sample)
XREP = 16  # xyT columns replicated 16x so proj matmul fills all 128 out rows
WROW = XREP * 2 * BL + D_OUT  # merged per-k row: [xyT_k tiled (128) | W^T_k (2048)]
NWARM = 12  # PE warm-up matmuls (cover the W-load ramp at LOW/MID clock)
EPS = 1e-12  # reference eps guard; norms here are O(500) so the guard is a no-op

KEEP = 12  # rows kept in the full-partition main DMA for samples 1-3
SLOW_LO = 120  # engine 79's partition block [120:128)
EX_BASE = {1: 0, 2: 32, 3: 64}  # rotating 32-partition group for stolen rows

_cache = {}


def _build_nc():
    import concourse.bass as bass  # noqa: F401
    import concourse.mybir as mybir
    import concourse.tile as tile
    from concourse import bacc
    from concourse.masks import make_identity

    f32 = mybir.dt.float32
    bf16 = mybir.dt.bfloat16
    nc = bacc.Bacc()

    wtx_ext = nc.declare_dram_parameter("WTX", [P, KC * WROW], bf16, isOutput=False)
    out_ext = nc.declare_dram_parameter("out", [BL, D_OUT * D_OUT], bf16, isOutput=True)

    # out flat index ((16p + u)*2048 + j): partition p's 16 rows are one 64KB run
    out4 = out_ext[:].rearrange("b (p u j) -> b p u j", p=P, u=U, j=D_OUT)
    # (u, p)-ordered view for the stolen-row DMA (SBUF partition m = u'*8 + q)
    out_ex = out_ext[:].rearrange("b (p u j) -> b u p j", p=P, u=U, j=D_OUT)
    wtx_r = wtx_ext[:].rearrange("p (k w) -> p k w", k=KC, w=WROW)

    with tile.TileContext(nc) as tc:
        with (
            tc.tile_pool(name="const", bufs=1) as const_pool,
            tc.tile_pool(name="persist", bufs=1) as persist,
            tc.tile_pool(name="small_psum", bufs=2, space="PSUM") as small_psum,
            tc.tile_pool(name="mm_psum", bufs=1, space="PSUM") as mm_psum,
            tc.tile_pool(name="ypb", bufs=1) as ypb_pool,
            tc.tile_pool(name="ypb_psum", bufs=2, space="PSUM") as ypb_psum,
            tc.tile_pool(name="outp", bufs=2) as out_pool,
            tc.tile_pool(name="exp", bufs=2) as ex_pool,
        ):
            # warm-up operand first so the PE chain starts ASAP
            warm = const_pool.tile([P, 512], bf16)
            nc.gpsimd.memset(warm[:], 0.125)

            ident8f = const_pool.tile([2 * BL, 2 * BL], f32)
            make_identity(nc, ident8f[:])
            ident8b = const_pool.tile([2 * BL, 2 * BL], bf16)
            make_identity(nc, ident8b[:])
            ident1 = const_pool.tile([1, 1], f32)
            nc.gpsimd.memset(ident1[:], 1.0)
            ones1 = const_pool.tile([1, P], f32)
            nc.gpsimd.memset(ones1[:], 1.0)
            # mask8[k, b, :] = 1.0 where k == BL + b else 0 - selects the yp
            # row of xy_proj in the K=8 broadcast matmul below.
            mask8 = const_pool.tile([2 * BL, BL, P], bf16)
            nc.gpsimd.memset(mask8[:], 0.0)
            nc.gpsimd.affine_select(
                out=mask8[:],
                in_=mask8[:],
                compare_op=mybir.AluOpType.not_equal,
                fill=1.0,
                base=-BL,
                pattern=[[-1, BL], [0, P]],
                channel_multiplier=1,
            )

            # pre-load the ACT sqrt table off the critical path
            sqwarm = const_pool.tile([1, 1], f32)
            nc.scalar.sqrt(sqwarm[:], ident1[:])

            # ---- input load: 5 chunked DMAs (k0 alone so matmuls start
            # early).  All bulk DMAs stay on the sync HWDGE queue. ----
            wtx = persist.tile([P, KC, WROW], bf16)
            for lo, hi in ((0, 1), (1, 2), (2, 4), (4, 6), (6, 8)):
                nc.sync.dma_start(wtx[:, lo:hi, :], wtx_r[:, lo:hi, :])

            # ---- PE warm-up: back-to-back dummy matmuls during the W load
            # keep the tensor engine clock ramping up ----
            psw = ypb_psum.tile([P, 512], f32, name="psw", tag="yp")
            for _ in range(NWARM):
                nc.tensor.matmul(psw[:], warm[:, 0:P], warm[:], start=True, stop=True)

            # ---- proj matmuls chase the chunk DMAs (k outer, o inner).
            # lhsT columns are host-replicated 16x (M=128): the PSUM result
            # has proj row r on partitions r, r+8, ..., so the big cast and
            # square ops below run 128-partition-wide (DVE perf mode). ----
            psxy = mm_psum.tile([P, OC, 512], f32, name="psxy", tag="mm")
            for k in range(KC):
                for o in range(OC):
                    nc.tensor.matmul(
                        psxy[:, o, :],
                        wtx[:, k, 0:P],
                        wtx[:, k, P + o * 512 : P + (o + 1) * 512],
                        start=(k == 0),
                        stop=(k == KC - 1),
                    )

            # cast PSUM->bf16 in two parallel halves (DVE + ACT)
            xy_proj = persist.tile([P, OC, 512], bf16)
            nc.vector.tensor_copy(xy_proj[:, 0:2, :], psxy[:, 0:2, :])
            nc.scalar.copy(xy_proj[:, 2:4, :], psxy[:, 2:4, :])
            xyp = xy_proj[:].rearrange("r o f -> r (o f)")

            # fused square + row-sum off the cast (ss = sum xyp^2); bf16 out
            # keeps the DVE multiply on the fast path, accum stays f32
            sqs = persist.tile([P, D_OUT // 2], bf16)
            ss2 = persist.tile([P, 2], f32)
            for h in range(2):
                xyph = xyp[:, h * (D_OUT // 2) : (h + 1) * (D_OUT // 2)]
                nc.vector.scalar_tensor_tensor(
                    out=sqs[:],
                    in0=xyph,
                    scalar=1.0,
                    in1=xyph,
                    op0=mybir.AluOpType.mult,
                    op1=mybir.AluOpType.mult,
                    accum_out=ss2[:, h : h + 1],
                )
            ss = persist.tile([P, 1], f32)
            nc.vector.tensor_tensor(
                ss[:], ss2[:, 0:1], ss2[:, 1:2], mybir.AluOpType.add
            )

            # ---- ypb[b] = yp_b broadcast to 128 partitions via K=8 masked PE
            # matmuls (plain casts; the norm scale rides in the fill ops).
            # ypb0 reuses the 4 psxy banks freed by the cast. ----
            ypb_tiles = [None] * BL
            ypb0 = ypb_pool.tile([P, D_OUT], bf16, name="ypb0", tag="ypb0")
            for j in range(4):
                nc.tensor.matmul(
                    psxy[:, j, :],
                    mask8[:, 0, :],
                    xy_proj[0 : 2 * BL, j, :],
                    start=True,
                    stop=True,
                )
            # plain casts (UNscaled - b=0 tiles carry s_0 in the fill's second
            # scalar slot), so ypb0 never waits on the norm chain.
            for j in range(2):
                nc.scalar.copy(ypb0[:, j * 512 : (j + 1) * 512], psxy[:, j, :])
            ypb_tiles[0] = ypb0

            # ---- xpi[p, u, b] = xp[b, 16p + u] via strided PE transposes of
            # xy_proj rows 0-3 (unscaled; one multi-slice PSUM tile so the 16
            # transposes run back-to-back without WAR stalls) ----
            xyp_r = xyp.rearrange("r (m u) -> u r m", m=P, u=U)
            xpi = persist.tile([P, U, BL], f32)
            ps16 = ypb_psum.tile([P, U, BL], bf16, name="ps16", tag="yp")
            for u in range(U):
                nc.tensor.transpose(
                    ps16[:, u, :], xyp_r[u, 0:BL, :], ident8b[0:BL, 0:BL]
                )

            # ---- norm chain: s_b = 1/sqrt(ssx_b*ssy_b) (norms ~O(500), the
            # reference eps guard can never bind for these inputs), then
            # sbc[:, b] = s_b broadcast to all 128 partitions via K=1 matmul ----
            ps_ss = small_psum.tile([1, 2 * BL], f32, name="ps_ss", tag="sp")
            nc.tensor.transpose(ps_ss[:], ss[0 : 2 * BL, :], ident8f[:])
            ps_sbc = small_psum.tile([P, BL], f32, name="ps_sbc", tag="sp")

            # DVE: ssT/nprod; ACT: sqrt; DVE: recip; PE: sbc broadcast
            ssT = persist.tile([1, 2 * BL], f32)
            nc.vector.tensor_copy(ssT[:], ps_ss[:])
            nprod = persist.tile([1, BL], f32)
            nc.vector.tensor_tensor(
                nprod[:], ssT[:, 0:BL], ssT[:, BL : 2 * BL], mybir.AluOpType.mult
            )
            nsqrt = persist.tile([1, BL], f32)
            nc.scalar.sqrt(nsqrt[:], nprod[:])
            sT = persist.tile([1, BL], f32)
            nc.vector.reciprocal(sT[:], nsqrt[:])
            nc.tensor.matmul(ps_sbc[:], ones1[:], sT[:], start=True, stop=True)
            sbc = persist.tile([P, BL], f32)
            nc.vector.tensor_copy(sbc[:], ps_sbc[:])

            # remaining ypb0 casts ride after sqrt on ACT (MMs long done)
            for j in range(2, 4):
                nc.scalar.copy(ypb0[:, j * 512 : (j + 1) * 512], psxy[:, j, :])

            # xpi copies: first half on DVE (feeds the first tile), rest ACT
            for u in range(U):
                if u < U // 2:
                    nc.vector.tensor_copy(xpi[:, u, :], ps16[:, u, :])
                else:
                    nc.scalar.copy(xpi[:, u, :], ps16[:, u, :])

            # (r, u, q) view of the proj tail covering o in [1920:2048):
            # o = 16*(120+q) + u, so element (u, q) of row r = xp_r[row 16p+u]
            # for p = 120+q -- exactly the stolen-row scalars.
            xyp_tail = xyp[:, SLOW_LO * U :].rearrange(
                "r (q u) -> r u q", q=P - SLOW_LO, u=U
            )

            def build_ypb(b):
                ypb = ypb_pool.tile([P, D_OUT], bf16, name=f"ypb{b}", tag=f"ypb{b}")
                for j in range(4):
                    psb = ypb_psum.tile([P, 512], f32, name="psb", tag="yp")
                    nc.tensor.matmul(
                        psb[:],
                        mask8[:, b, :],
                        xy_proj[0 : 2 * BL, j, :],
                        start=True,
                        stop=True,
                    )
                    if j % 2 == 0:
                        nc.vector.tensor_scalar_mul(
                            ypb[:, j * 512 : (j + 1) * 512], psb[:], sbc[:, b : b + 1]
                        )
                    else:
                        nc.scalar.mul(
                            ypb[:, j * 512 : (j + 1) * 512], psb[:], sbc[:, b : b + 1]
                        )
                ypb_tiles[b] = ypb

            # ---- outer products: 8MB whole-sample bf16 tiles, stream out ----
            for b in range(BL):
                if b >= 1:
                    build_ypb(b)
                ot = out_pool.tile([P, U, D_OUT], bf16, name="ot")
                first = b == 0
                for u in range(U):
                    if b == 0:
                        # b=0 tile: all-DVE, dual-scalar (ypb0 unscaled):
                        # ot = (ypb0 * xp_i) * s_0 - nothing waits on ACT
                        nc.vector.tensor_scalar(
                            out=ot[:, u, :],
                            in0=ypb_tiles[b][:],
                            scalar1=xpi[:, u, b : b + 1],
                            scalar2=sbc[:, b : b + 1],
                            op0=mybir.AluOpType.mult,
                            op1=mybir.AluOpType.mult,
                        )
                    elif u % 4 != 3:
                        # later tiles: 12 DVE + 4 ACT fills
                        nc.vector.tensor_scalar_mul(
                            ot[:, u, :], ypb_tiles[b][:], xpi[:, u, b : b + 1]
                        )
                    else:
                        nc.scalar.mul(
                            ot[:, u, :], ypb_tiles[b][:], xpi[:, u, b : b + 1]
                        )
                    # first tile streams out early in small pieces
                    if first and u in (1, 3, 7):
                        lo = {1: 0, 3: 2, 7: 4}[u]
                        nc.sync.dma_start(
                            out4[b][:, lo : u + 1], ot[:, lo : u + 1, :]
                        )
                if first:
                    nc.sync.dma_start(out4[b][:, 8:U], ot[:, 8:U, :])
                    continue

                # --- stolen rows of partitions 120-127 (engine 79's block):
                # rank-1 masked PE matmuls xst_b (x) yp_b into a rotating
                # 32-partition PSUM group; scale is already inside ypb. ---
                eb = EX_BASE[b]
                nu = U - KEEP
                xst = persist.tile([2 * BL, nu, P - SLOW_LO], bf16, name=f"xst{b}")
                nc.vector.tensor_scalar_mul(
                    xst[:], xyp_tail[0 : 2 * BL, KEEP:U, :], ident8f[:, b : b + 1]
                )
                psex = mm_psum.tile([P, OC, 512], f32, name=f"psex{b}", tag="mm")
                for o in range(OC):
                    nc.tensor.matmul(
                        psex[eb : eb + 32, o, :],
                        xst[:],
                        ypb_tiles[b][0 : 2 * BL, o * 512 : (o + 1) * 512],
                        start=True,
                        stop=True,
                    )
                ex = ex_pool.tile([P, D_OUT], bf16, name="ex")
                nc.scalar.copy(
                    ex[eb : eb + 32, :].rearrange("m (o f) -> m o f", o=OC, f=512),
                    psex[eb : eb + 32, :, :],
                )

                # main slab: all 128 partitions, u < KEEP (engine 79's whole
                # share), on the sync HWDGE queue
                nc.sync.dma_start(out4[b][:, 0:KEEP], ot[:, 0:KEEP, :])
                # fast partitions' tail via SWDGE: deals per-partition, so it
                # spreads over engines 0-14 and skips engine 79
                nc.gpsimd.dma_start(
                    out4[b, 0:SLOW_LO, KEEP:U], ot[0:SLOW_LO, KEEP:U, :]
                )
                # stolen rows from the rotating fast-port group (SBUF
                # partition eb + u'*8 + q <-> DRAM row 16*(120+q) + KEEP + u')
                nc.gpsimd.dma_start(
                    out_ex[b, KEEP:U, SLOW_LO:P, :], ex[eb : eb + 32, :]
                )

    nc.compile()
    return nc


def _get_nc():
    if "nc" not in _cache:
        _cache["nc"] = _build_nc()
    return _cache["nc"]


def _prep_in_maps(x, y, W):
    """Host-side prep: bf16 conversion + merged SBUF-ready transposed layout.

    WTX[p, k*WROW + 0:128]   = concat(x_shard, y_shard).T[k*128 + p, :] tiled 16x
    WTX[p, k*WROW + 128:]    = W.T[k*128 + p, :]
    """
    import ml_dtypes

    bf = ml_dtypes.bfloat16
    x = np.ascontiguousarray(x, dtype=np.float32)
    y = np.ascontiguousarray(y, dtype=np.float32)
    W = np.ascontiguousarray(W, dtype=np.float32)

    wt = W.astype(bf).T.reshape(KC, P, D_OUT)  # [k, p, o]
    in_maps = []
    for c in range(NCORES):
        xy = np.concatenate(
            [x[c * BL : (c + 1) * BL], y[c * BL : (c + 1) * BL]], axis=0
        ).astype(bf)  # [8, 1024]
        xyt = np.tile(xy.T.reshape(KC, P, 2 * BL), (1, 1, XREP))  # [k, p, 128]
        merged = np.concatenate([xyt, wt], axis=2)  # [k, p, 128+2048]
        in_maps.append(
            {"WTX": np.ascontiguousarray(merged.transpose(1, 0, 2).reshape(P, KC * WROW))}
        )
    return in_maps


def _bf16_to_f32(a):
    return (a.view(np.uint16).astype(np.uint32) << 16).view(np.float32)


def kernel(x: np.ndarray, y: np.ndarray, W: np.ndarray) -> np.ndarray:
    from concourse.bass_utils import run_bass_kernel_spmd

    nc = _get_nc()
    in_maps = _prep_in_maps(x, y, W)
    res = run_bass_kernel_spmd(nc, in_maps, list(range(NCORES))).results
    o16 = np.concatenate([np.asarray(res[c]["out"]) for c in range(NCORES)], axis=0)
    return _bf16_to_f32(np.ascontiguousarray(o16))
